# revision 16
# baseline (speedup 1.0000x reference)
"""Trainium2 Bass kernel for nn_BatchedMHC (moe_routing).

Sharding: data-parallel on B across 8 cores (each core handles one batch).
Per core: RMSNorm-scaled gate matmuls (pre/post sigmoid gates + res 4x4
Sinkhorn-normalized gates) for 8 active experts over 4096 tokens.

All expert gathering / weight folding (norm_w, alpha, sign flips, chunk
permutation) happens on host in numpy; the device kernel sees pre-folded
bf16 weight tensors.
"""

import sys
from contextlib import ExitStack

import numpy as np

try:
    import concourse.bass as bass  # noqa: F401
except ImportError:
    sys.path.insert(0, "/opt/trn_rl_repo")

import concourse.bass as bass
import concourse.tile as tile
from concourse import bacc, mybir
from concourse.bass_utils import run_bass_kernel_spmd

import ml_dtypes

_BF16 = np.dtype(ml_dtypes.bfloat16)


def _ensure_axon_hooks():
    """Provide antenv.axon_hooks if the image lacks it (needed only when
    BASS_TRACE=1 asks run_bass_kernel_spmd for an NTFF profile)."""
    try:
        import antenv.axon_hooks  # noqa: F401
        return
    except ImportError:
        pass
    import types

    mod = types.ModuleType("antenv.axon_hooks")
    holder = {"hook": None}
    mod.set_axon_ntff_profile_hook = lambda f: holder.__setitem__("hook", f)
    mod.get_axon_ntff_profile_hook = lambda: holder["hook"]
    sys.modules["antenv.axon_hooks"] = mod
    try:
        from trn_agent_boot.trn_boot import _ntff_profile_via_ctypes

        mod.set_axon_ntff_profile_hook(
            _ntff_profile_via_ctypes("/opt/axon/libaxon_pjrt.so")
        )
    except Exception:
        pass


_ensure_axon_hooks()

F32 = mybir.dt.float32
F32R = mybir.dt.float32r
BF16 = mybir.dt.bfloat16
I32 = mybir.dt.int32
AF = mybir.ActivationFunctionType
ALU = mybir.AluOpType

# Problem constants (hardcoded per spec)
N_EXPERTS, D, NS = 64, 512, 4
K, B, T = 8, 8, 4096
ND = NS * D  # 2048
NCORES = 8
NCH = ND // 128  # 16 contraction chunks
MMB = 512        # matmul token block (one PSUM bank)
SUP = 2048       # sinkhorn super-block (4 sub-blocks of 512)
NSUP = T // SUP  # 2
NTT = SUP // 128  # 16 token-tiles per super
G_RES = K * 16    # 128 res gates
G_PP = K * NS * 2  # 64 pre+post gates
LN2 = 0.6931471805599453
MAGIC_P1 = 0x5F3759E0  # 0x5f3759df + 1 (two's-complement rsub trick)

_CACHE = {}


def _build_nc():
    nc = bacc.Bacc(None, target_bir_lowering=False)

    x_in = nc.declare_dram_parameter("x", [NS, T, D], F32, isOutput=False)
    w_res = nc.declare_dram_parameter("w_res", [NCH, 128, G_RES], BF16, isOutput=False)
    w_pp = nc.declare_dram_parameter("w_pp", [NCH, 128, G_PP], BF16, isOutput=False)
    b_res = nc.declare_dram_parameter("b_res", [G_RES, 1], F32, isOutput=False)
    b_pp = nc.declare_dram_parameter("b_pp", [G_PP, 1], F32, isOutput=False)
    addc_pp = nc.declare_dram_parameter("addc_pp", [G_PP, 1], F32, isOutput=False)
    ind_r = nc.declare_dram_parameter("ind_r", [128, 32], BF16, isOutput=False)
    ind_c = nc.declare_dram_parameter("ind_c", [128, 32], BF16, isOutput=False)
    ind_bu = nc.declare_dram_parameter("ind_bu", [128, 128], BF16, isOutput=False)
    ind_bv = nc.declare_dram_parameter("ind_bv", [128, 128], BF16, isOutput=False)
    ident = nc.declare_dram_parameter("ident", [128, 128], F32, isOutput=False)
    ones_row = nc.declare_dram_parameter("ones_row", [1, 128], BF16, isOutput=False)

    out_res = nc.declare_dram_parameter("out_res", [T, G_RES], F32, isOutput=True)
    out_pp = nc.declare_dram_parameter("out_pp", [T, G_PP], F32, isOutput=True)

    with tile.TileContext(nc) as tc, ExitStack() as ctx:
        # ---- constants (loaded once) ----
        cpool = ctx.enter_context(tc.tile_pool(name="consts", bufs=1))
        wres_sb = cpool.tile([128, NCH, G_RES], BF16, tag="wres")
        wpp_sb = cpool.tile([128, NCH, G_PP], BF16, tag="wpp")
        bres_sb = cpool.tile([G_RES, 1], F32, tag="bres")
        bpp_sb = cpool.tile([G_PP, 1], F32, tag="bpp")
        addc_sb = cpool.tile([G_PP, 1], F32, tag="addc")
        r_sb = cpool.tile([128, 32], BF16, tag="indr")
        c_sb = cpool.tile([128, 32], BF16, tag="indc")
        bu_sb = cpool.tile([128, 128], BF16, tag="indbu")
        bv_sb = cpool.tile([128, 128], BF16, tag="indbv")
        id_sb = cpool.tile([128, 128], F32, tag="ident")
        ones_sb = cpool.tile([1, 128], BF16, tag="ones")

        nc.gpsimd.dma_start(out=wres_sb[:], in_=w_res.rearrange("c p g -> p c g"))
        nc.gpsimd.dma_start(out=wpp_sb[:], in_=w_pp.rearrange("c p g -> p c g"))
        nc.gpsimd.dma_start(out=bres_sb[:], in_=b_res[:])
        nc.gpsimd.dma_start(out=bpp_sb[:], in_=b_pp[:])
        nc.gpsimd.dma_start(out=addc_sb[:], in_=addc_pp[:])
        nc.gpsimd.dma_start(out=r_sb[:], in_=ind_r[:])
        nc.gpsimd.dma_start(out=c_sb[:], in_=ind_c[:])
        nc.gpsimd.dma_start(out=bu_sb[:], in_=ind_bu[:])
        nc.gpsimd.dma_start(out=bv_sb[:], in_=ind_bv[:])
        nc.gpsimd.dma_start(out=id_sb[:], in_=ident[:])
        nc.gpsimd.dma_start(out=ones_sb[:], in_=ones_row[:])

        # ---- working pools ----
        xnat = ctx.enter_context(tc.tile_pool(name="xnat", bufs=3))
        xtp = ctx.enter_context(tc.tile_pool(name="xt", bufs=2))
        sq_scr = ctx.enter_context(tc.tile_pool(name="sqscr", bufs=2))
        stat = ctx.enter_context(tc.tile_pool(name="stat", bufs=1))
        rchain = ctx.enter_context(tc.tile_pool(name="rchain", bufs=2))
        m0p = ctx.enter_context(tc.tile_pool(name="m0", bufs=2))
        tmpp = ctx.enter_context(tc.tile_pool(name="tmp", bufs=2))
        uvp = ctx.enter_context(tc.tile_pool(name="uv", bufs=2))
        rmsbp = ctx.enter_context(tc.tile_pool(name="rmsb", bufs=2))
        eppp = ctx.enter_context(tc.tile_pool(name="epp", bufs=2))
        mzp = ctx.enter_context(tc.tile_pool(name="mz", bufs=2))
        onat = ctx.enter_context(tc.tile_pool(name="onat", bufs=2))

        zres_ps = ctx.enter_context(tc.tile_pool(name="zres", bufs=1, space="PSUM"))
        zpp_ps = ctx.enter_context(tc.tile_pool(name="zpp", bufs=1, space="PSUM"))
        rw_ps = ctx.enter_context(tc.tile_pool(name="rwps", bufs=1, space="PSUM"))
        u_ps = ctx.enter_context(tc.tile_pool(name="ups", bufs=1, space="PSUM"))
        t_ps = ctx.enter_context(tc.tile_pool(name="tps", bufs=1, space="PSUM"))

        # persistent stats
        ss_sb = stat.tile([128, T // 128], F32, tag="ss")   # per-token sum of squares

        HSUP = 1024  # load/matmul pipeline granularity (half a sinkhorn super)

        for s in range(NSUP):  # sinkhorn super-blocks of 2048 tokens
            t0 = s * SUP
            m0 = m0p.tile([128, SUP], BF16, tag="m0")
            epp = eppp.tile([G_PP, SUP], F32, tag="epp")

            for hs in range(SUP // HSUP):  # half-supers of 1024
                h0 = t0 + hs * HSUP
                NH = HSUP // 128  # 8 token-tiles
                # ---------- load + ss + transpose ----------
                xt_blks = []
                for m in range(HSUP // MMB):
                    xt_blk = xtp.tile([128, NCH, MMB], BF16, tag="xt")
                    for j in range(MMB // 128):
                        jt = (h0 + m * MMB) // 128 + j
                        xn = xnat.tile([128, ND], BF16, tag="xn")
                        nc.gpsimd.dma_start(
                            out=xn[:],
                            in_=x_in[:, jt * 128:(jt + 1) * 128, :].rearrange(
                                "s t d -> t s d"
                            ),
                        )
                        scr = sq_scr.tile([128, ND], BF16, tag="sq")
                        nc.scalar.activation(
                            scr[:], xn[:], AF.Square,
                            accum_out=ss_sb[:, jt:jt + 1],
                        )
                        # xbar transpose: chunk c, row p holds d = c*128 + p
                        nc.sync.dma_start_transpose(
                            out=xt_blk[:, :, j * 128:(j + 1) * 128], in_=xn[:]
                        )
                    xt_blks.append(xt_blk)

                # ---------- rms = rsqrt(mean(x^2)+eps), pure-DVE Newton ----------
                ssl = ss_sb[:, h0 // 128:(h0 + HSUP) // 128]
                msf = rchain.tile([128, NH], F32, tag="msf")
                nc.vector.tensor_scalar(
                    out=msf[:], in0=ssl, scalar1=1.0 / ND, scalar2=1e-8,
                    op0=ALU.mult, op1=ALU.add,
                )
                yi = rchain.tile([128, NH], I32, tag="yi")
                nc.vector.tensor_scalar(
                    out=yi[:], in0=msf[:].bitcast(I32), scalar1=1, scalar2=-1,
                    op0=ALU.logical_shift_right, op1=ALU.bitwise_xor,
                )
                nc.vector.tensor_scalar(
                    out=yi[:], in0=yi[:], scalar1=MAGIC_P1, scalar2=None,
                    op0=ALU.add,
                )
                y = yi[:].bitcast(F32)
                t1 = rchain.tile([128, NH], F32, tag="t1")
                t2 = rchain.tile([128, NH], F32, tag="t2")
                for _ in range(2):
                    nc.vector.tensor_tensor(out=t1[:], in0=y, in1=y, op=ALU.mult)
                    nc.vector.tensor_tensor(
                        out=t2[:], in0=t1[:], in1=msf[:], op=ALU.mult
                    )
                    nc.vector.tensor_scalar(
                        out=t2[:], in0=t2[:], scalar1=-0.5, scalar2=1.5,
                        op0=ALU.mult, op1=ALU.add,
                    )
                    nc.vector.tensor_tensor(out=t1[:], in0=y, in1=t2[:], op=ALU.mult)
                    nc.vector.tensor_copy(out=y, in_=t1[:])
                # transpose rms [128, 8] -> partition-0 row [1, 8*128] so token
                # runs are contiguous (feeds the ones-broadcast matmul)
                rmsT_ps = t_ps.tile([1, NH, 128], F32, tag="tps")
                for j in range(NH):
                    nc.tensor.transpose(
                        rmsT_ps[0:1, j, :], in_=yi[:, j:j + 1].bitcast(F32),
                        identity=id_sb[:],
                    )
                rmsT = rchain.tile([1, NH, 128], BF16, tag="rmsT")
                nc.scalar.copy(out=rmsT[:], in_=rmsT_ps[:])

                # ---------- gate matmuls + rms scaling + exp ----------
                for m in range(HSUP // MMB):
                    toff = h0 + m * MMB
                    soff = toff - t0
                    xt_blk = xt_blks[m]
                    zres = zres_ps.tile([128, MMB], F32, tag="zres")
                    zpp = zpp_ps.tile([G_PP, MMB], F32, tag="zpp")
                    for c in range(NCH):
                        nc.tensor.matmul(
                            zres[:], lhsT=wres_sb[:, c, :], rhs=xt_blk[:, c, :],
                            start=(c == 0), stop=(c == NCH - 1),
                        )
                    for c in range(NCH):
                        nc.tensor.matmul(
                            zpp[:], lhsT=wpp_sb[:, c, :], rhs=xt_blk[:, c, :],
                            start=(c == 0), stop=(c == NCH - 1),
                        )
                    # broadcast rms over partitions: ones[1,128].T @ rms[1,128]
                    rmsb_ps = rw_ps.tile([128, MMB], F32, tag="rwps")
                    for j in range(MMB // 128):
                        jr = m * (MMB // 128) + j
                        nc.tensor.matmul(
                            rmsb_ps[:, j * 128:(j + 1) * 128], lhsT=ones_sb[:],
                            rhs=rmsT[0:1, jr, :],
                            start=True, stop=True,
                        )
                    rmsb = rmsbp.tile([128, MMB], BF16, tag="rmsb")
                    nc.scalar.copy(out=rmsb[:], in_=rmsb_ps[:])
                    zs = tmpp.tile([128, MMB], F32, tag="zs")
                    nc.vector.tensor_tensor(
                        out=zs[:], in0=zres[:], in1=rmsb[:], op=ALU.mult
                    )
                    nc.scalar.activation(
                        m0[:, soff:soff + MMB], zs[:], AF.Exp, bias=bres_sb[:],
                    )
                    zsp = tmpp.tile([G_PP, MMB], F32, tag="zsp")
                    nc.vector.tensor_tensor(
                        out=zsp[:], in0=zpp[:], in1=rmsb[0:G_PP, :], op=ALU.mult
                    )
                    nc.scalar.activation(
                        epp[:, soff:soff + MMB], zsp[:], AF.Exp, bias=bpp_sb[:],
                    )

            # ---------- pre/post sigmoid tail ----------
            nc.scalar.activation(epp[:], epp[:], AF.Identity, bias=addc_sb[:])
            hpp = eppp.tile([G_PP, SUP], F32, tag="hpp")
            nc.vector.reciprocal_approx_fast(out=hpp[:], in_=epp[:])

            # ---------- sinkhorn, u/v form ----------
            # M0 [128=(k,r,c), 2048]; u/v packed [128=(q,kr), 512] (q = token
            # sub-block); broadcasts/reduces on PE via 32-wide tile packing.
            def bcast(uv_sb, ind):
                halves = []
                for h in range(2):
                    ubh = u_ps.tile([128, 1024], F32, tag="ub")
                    for qq in range(2):
                        q = h * 2 + qq
                        nc.tensor.matmul(
                            ubh[:, qq * MMB:(qq + 1) * MMB],
                            lhsT=ind[q * 32:(q + 1) * 32, :],
                            rhs=uv_sb[q * 32:(q + 1) * 32, :],
                            start=True, stop=True, tile_position=(q * 32, 0),
                        )
                    halves.append(ubh)
                return halves

            def group_sum(src, ind):
                wsum = rw_ps.tile([128, MMB], F32, tag="rwps")
                for q in range(4):
                    nc.tensor.matmul(
                        wsum[q * 32:(q + 1) * 32, :], lhsT=ind,
                        rhs=src[:, q * MMB:(q + 1) * MMB],
                        start=True, stop=True, tile_position=(0, q * 32),
                    )
                return wsum

            def mul_halves(halves, out_dtype, tag):
                o = (mzp if out_dtype == F32 else tmpp).tile(
                    [128, SUP], out_dtype, tag=tag
                )
                for h in range(2):
                    sl = slice(h * 1024, (h + 1) * 1024)
                    nc.vector.tensor_tensor(
                        out=o[:, sl], in0=m0[:, sl] if tag != "mzf" else prod[:, sl],
                        in1=halves[h][:], op=ALU.mult,
                    )
                return o

            vhalves = None
            prod = None
            for it in range(6):
                # row step
                rs_src = m0 if it == 0 else mul_halves(vhalves, BF16, "prod")
                wsum = group_sum(rs_src, r_sb[:])
                u = uvp.tile([128, MMB], F32, tag="uv")
                nc.vector.reciprocal_approx_fast(out=u[:], in_=wsum[:])
                ub16 = uvp.tile([128, MMB], BF16, tag="uvb")
                nc.vector.tensor_copy(out=ub16[:], in_=u[:])
                uhalves = bcast(ub16[:], bu_sb)
                # col step
                prod = mul_halves(uhalves, BF16, "prod")
                wsum = group_sum(prod, c_sb[:])
                v = uvp.tile([128, MMB], F32, tag="uv")
                nc.vector.reciprocal_approx_fast(out=v[:], in_=wsum[:])
                vb16 = uvp.tile([128, MMB], BF16, tag="uvb")
                nc.vector.tensor_copy(out=vb16[:], in_=v[:])
                vhalves = bcast(vb16[:], bv_sb)

            # final M = (M0*U6) * V6 ; prod = M0*U6 already
            mz = mul_halves(vhalves, F32, "mzf")

            # ---------- naturalize via PE transpose + store ----------
            o_res = onat.tile([128, NTT, G_RES], F32, tag="ores")
            o_pp = onat.tile([128, NTT, G_PP], F32, tag="opp")
            for g in range(4):
                tp = t_ps.tile([128, 4, 128], F32, tag="tps")
                for j in range(4):
                    jt = g * 4 + j
                    nc.tensor.transpose(
                        tp[:, j, :], in_=mz[:, jt * 128:(jt + 1) * 128],
                        identity=id_sb[:],
                    )
                nc.scalar.copy(out=o_res[:, g * 4:(g + 1) * 4, :], in_=tp[:])
                tpp = t_ps.tile([128, 4, 128], F32, tag="tps")
                for j in range(4):
                    jt = g * 4 + j
                    nc.tensor.transpose(
                        tpp[:, j, 0:G_PP], in_=hpp[:, jt * 128:(jt + 1) * 128],
                        identity=id_sb[0:G_PP, 0:G_PP],
                    )
                nc.scalar.copy(
                    out=o_pp[:, g * 4:(g + 1) * 4, :], in_=tpp[:, :, 0:G_PP]
                )
            nc.sync.dma_start(
                out=out_res[t0:t0 + SUP, :].rearrange("(j p) g -> p j g", p=128),
                in_=o_res[:],
            )
            nc.sync.dma_start(
                out=out_pp[t0:t0 + SUP, :].rearrange("(j p) g -> p j g", p=128),
                in_=o_pp[:],
            )

    nc.finalize()
    return nc


def _host_prep(inputs):
    idx = np.asarray(inputs["active_idx"]).astype(np.int64)
    nw = np.asarray(inputs["norm_w"], np.float32)[idx]  # (K, ND)
    a_pre = np.asarray(inputs["alpha_pre"], np.float32)[idx]
    a_post = np.asarray(inputs["alpha_post"], np.float32)[idx]
    a_res = np.asarray(inputs["alpha_res"], np.float32)[idx]
    wp = np.asarray(inputs["phi_pre_w"], np.float32)[idx] * nw[:, None, :]
    wq = np.asarray(inputs["phi_post_w"], np.float32)[idx] * nw[:, None, :]
    wr = np.asarray(inputs["phi_res_w"], np.float32)[idx] * nw[:, None, :]
    bp = np.asarray(inputs["b_pre"], np.float32)[idx]
    bq = np.asarray(inputs["b_post"], np.float32)[idx]
    br = np.asarray(inputs["b_res"], np.float32)[idx]

    w_res = (wr * a_res[:, None, None]).reshape(G_RES, ND)
    w_pre = (-wp * a_pre[:, None, None]).reshape(K * NS, ND)
    w_post = (-wq * a_post[:, None, None]).reshape(K * NS, ND)
    w_pp = np.concatenate([w_pre, w_post], 0)

    def chunked(w):  # (G, ND) -> (NCH, 128, G); chunk c row p holds d=c*128+p
        return np.ascontiguousarray(w.reshape(-1, NCH, 128).transpose(1, 2, 0))

    b_res_d = br.reshape(G_RES, 1)
    b_pp_d = np.concatenate(
        [-bp.reshape(-1), -bq.reshape(-1) - LN2]
    ).reshape(G_PP, 1)
    addc = np.concatenate(
        [np.full(K * NS, 1.0), np.full(K * NS, 0.5)]
    ).reshape(G_PP, 1)

    p = np.arange(128)
    kk, rr, cc = p // 16, (p // 4) % 4, p % 4
    ind_r = np.zeros((128, 32), np.float32)
    ind_r[p, kk * 4 + rr] = 1.0
    ind_c = np.zeros((128, 32), np.float32)
    ind_c[p, kk * 4 + cc] = 1.0
    ind_bu = np.tile(ind_r.T, (4, 1))  # [128,128], 4 stacked [32,128] blocks
    ind_bv = np.tile(ind_c.T, (4, 1))

    common = {
        "w_res": chunked(w_res).astype(_BF16),
        "w_pp": chunked(w_pp).astype(_BF16),
        "b_res": b_res_d.astype(np.float32),
        "b_pp": b_pp_d.astype(np.float32),
        "addc_pp": addc.astype(np.float32),
        "ind_r": ind_r.astype(_BF16),
        "ind_c": ind_c.astype(_BF16),
        "ind_bu": np.ascontiguousarray(ind_bu).astype(_BF16),
        "ind_bv": np.ascontiguousarray(ind_bv).astype(_BF16),
        "ident": np.eye(128, dtype=np.float32),
        "ones_row": np.ones((1, 128), _BF16),
    }
    stream = np.asarray(inputs["stream"], np.float32)
    in_maps = []
    for b in range(NCORES):
        m = dict(common)
        m["x"] = np.ascontiguousarray(stream[b])
        in_maps.append(m)
    return in_maps


def kernel(**inputs):
    if "nc" not in _CACHE:
        _CACHE["nc"] = _build_nc()
    nc = _CACHE["nc"]
    in_maps = _host_prep(inputs)
    res = run_bass_kernel_spmd(nc, in_maps, core_ids=list(range(NCORES)))
    _CACHE["last_res"] = res
    outs = res.results

    h_res = np.empty((K, B, T, NS, NS), np.float32)
    h_pre = np.empty((K, B, T, NS), np.float32)
    h_post = np.empty((K, B, T, NS), np.float32)
    for b in range(NCORES):
        o_res = np.asarray(outs[b]["out_res"])  # (T, 128)
        o_pp = np.asarray(outs[b]["out_pp"])    # (T, 64)
        h_res[:, b] = o_res.reshape(T, K, 4, 4).transpose(1, 0, 2, 3)
        h_pre[:, b] = o_pp[:, :K * NS].reshape(T, K, NS).transpose(1, 0, 2)
        h_post[:, b] = o_pp[:, K * NS:].reshape(T, K, NS).transpose(1, 0, 2)
    return h_res, h_pre, h_post
